# revision 20
# baseline (speedup 1.0000x reference)
"""Adaptive depthwise-conv kernel for Trainium2 (8 NeuronCores, SPMD).

Pipeline (all tensor compute on device):
  NEFF-A (sample-parallel, 2 samples/core): kernel-size predictor
    1x1 conv (C=256 -> 32, bf16 matmul) -> relu -> 1x1 conv (32 -> 1)
    -> relu -> per-chunk partial sums returned to host; host finishes the
    scalar mean/floor/clip to get the per-sample radius.
  NEFF-B (channel-parallel, 32 channels/core): depthwise 21x21 conv as
    banded-Toeplitz matmuls on TensorE. For each channel and kernel
    column kj, a [148,128] banded matrix (kernel column on 21 diagonals)
    contracts padded input rows; all 16 samples ride the matmul free dim;
    21 kj passes + packed-tail passes accumulate in PSUM.

Host work is limited to data movement (reflect padding, bf16 cast,
Toeplitz weight layout, DMA-friendly transposes) and the final scalar
floor/clip per sample. All DRAM layouts are chosen so each DMA descriptor
is a multi-KB contiguous run (keeps the 16 DMA queues efficient and the
PE HAM-warm).
"""

import numpy as np
import ml_dtypes

B, C, H, W, K, P = 16, 256, 128, 128, 21, 10
HP = H + 2 * P          # 148
NCORES = 8
CPC = C // NCORES       # 32 channels per core
SPC = B // NCORES       # 2 samples per core
NPIX = H * W
PCHUNK = 512            # predictor matmul free-dim chunk
QCHUNK = 2048           # predictor DMA chunk (4 matmul chunks)
NCHUNK = NPIX // PCHUNK
TAIL_PACK = 6           # kj's packed per tail tile (6*20 = 120 rows)
TROWS = 20 * TAIL_PACK  # 120

_BF16 = ml_dtypes.bfloat16


def _split_multiwaits(nc):
    """Hoist extra sem waits onto EventSemaphore nops.

    This walrus build rejects instructions carrying more than one sync
    wait ("Too many sync wait commands"); Tile emits up to ~3 per
    instruction and the tail drain carries one per live proc. Splitting
    is semantics-preserving: the same-engine EventSemaphores execute in
    program order before the instruction.
    """
    import concourse.mybir as mybir

    # Dedupe PE weight loads: consecutive matmuls (in final scheduled
    # order) with an identical stationary AP skip the 128-cycle reload.
    for f in nc.m.functions:
        for b in f.blocks:
            prev_key = None
            for inst in b.instructions:
                if isinstance(inst, mybir.InstMatmult):
                    key = repr(inst.ins[1])
                    if key == prev_key:
                        inst.ldweights = False
                    prev_key = key

    n = 0
    for f in nc.m.functions:
        for b in f.blocks:
            lst = b.instructions
            i = 0
            while i < len(lst):
                inst = lst[i]
                si = inst.sync_info
                if si and si.on_wait and len(si.on_wait) > 1:
                    waits = list(si.on_wait)
                    for w in waits[:-1]:
                        ev = mybir.InstEventSemaphore(
                            name=f"wsplit_{n}",
                            engine=inst.engine,
                            sync_info=mybir.SyncInfo(on_wait=[w], on_update=[]),
                            ins=[],
                            outs=[],
                            bass_nofuse=True,
                        )
                        lst.insert(i, ev)
                        n += 1
                        i += 1
                    si.on_wait = [waits[-1]]
                i += 1
    return nc


# ---------------------------------------------------------------- NEFF-A


def _build_pred():
    import concourse.bass as bass
    import concourse.mybir as mybir
    import concourse.tile as tile

    dt = mybir.dt
    nc = bass.Bass()
    xs = nc.declare_dram_parameter(
        "xs", [SPC, 2, 128, NPIX], dt.float8e4, isOutput=False
    )
    w1t = nc.declare_dram_parameter("w1t", [C, 32], dt.float8e4, isOutput=False)
    w2t = nc.declare_dram_parameter("w2t", [128, QCHUNK // PCHUNK], dt.bfloat16, isOutput=False)
    b1p = nc.declare_dram_parameter("b1p", [128, 1], dt.float32, isOutput=False)
    b2p = nc.declare_dram_parameter("b2p", [QCHUNK // PCHUNK, 1], dt.float32, isOutput=False)
    outp = nc.declare_dram_parameter(
        "out", [QCHUNK // PCHUNK, SPC * (NPIX // QCHUNK)], dt.float32,
        isOutput=True,
    )

    relu = mybir.ActivationFunctionType.Relu
    nsub = QCHUNK // PCHUNK       # 4 M=32 stripes packed per PSUM tile
    nqq = NPIX // QCHUNK          # big chunks per sample
    with tile.TileContext(nc) as tc:
        with (
            tc.tile_pool(name="const", bufs=1) as cpool,
            tc.tile_pool(name="x", bufs=6) as xpool,
            tc.tile_pool(name="h", bufs=4) as hpool,
            tc.tile_pool(name="z", bufs=4) as zpool,
            tc.tile_pool(name="sums", bufs=1) as spool,
            tc.tile_pool(name="ps1", bufs=4, space="PSUM") as ps1,
            tc.tile_pool(name="ps2", bufs=4, space="PSUM") as ps2,
        ):
            w1tile = cpool.tile([128, 2, 32], dt.float8e4)
            nc.sync.dma_start(
                w1tile[:], w1t.rearrange("(ck p) o -> p ck o", ck=2)
            )
            # block-diagonal 2nd-layer weights: w2blk[32s+o, s] = w2[o]
            w2tile = cpool.tile([128, nsub], dt.bfloat16)
            nc.sync.dma_start(w2tile[:], w2t[:, :])
            b1tile = cpool.tile([128, 1], dt.float32)   # b1 tiled 4x
            nc.sync.dma_start(b1tile[:], b1p[:, :])
            b2tile = cpool.tile([nsub, 1], dt.float32)
            nc.sync.dma_start(b2tile[:], b2p[:, :])
            sums = spool.tile([nsub, SPC * nqq], dt.float32)

            for s in range(SPC):
                for qq in range(nqq):
                    xl = xpool.tile([128, 2, QCHUNK], dt.float8e4)
                    nc.sync.dma_start(
                        xl[:],
                        xs[s, :, :, qq * QCHUNK : (qq + 1) * QCHUNK]
                        .rearrange("ck p pix -> p ck pix"),
                    )
                    # 4 output stripes [32, 512] at partition offsets 32*sub
                    ph = ps1.tile([128, PCHUNK], dt.float32)
                    for ck in range(2):
                        for sub in range(nsub):
                            c0 = sub * PCHUNK
                            nc.tensor.matmul(
                                ph[32 * sub : 32 * (sub + 1), :],
                                w1tile[:, ck, :],
                                xl[:, ck, c0 : c0 + PCHUNK],
                                start=(ck == 0),
                                stop=(ck == 1),
                                tile_position=(0, 32 * sub),
                            )
                    hs = hpool.tile([128, PCHUNK], dt.bfloat16)
                    nc.scalar.activation(hs[:], ph[:], relu, bias=b1tile[:])
                    p2 = ps2.tile([nsub, PCHUNK], dt.float32)
                    nc.tensor.matmul(
                        p2[:], w2tile[:], hs[:], start=True, stop=True
                    )
                    zr = zpool.tile([nsub, PCHUNK], dt.float32)
                    nc.scalar.activation(zr[:], p2[:], relu, bias=b2tile[:])
                    idx = s * nqq + qq
                    nc.vector.reduce_sum(
                        sums[:, idx : idx + 1], zr[:],
                        axis=mybir.AxisListType.X,
                    )
            nc.sync.dma_start(outp[:, :], sums[:])
    return _split_multiwaits(nc)


def _run_pred(x, w1, b1, w2, b2, trace=False):
    from concourse.bass_utils import run_bass_kernel_spmd

    nc = _build_pred()
    xf = np.ascontiguousarray(x.reshape(B, C, NPIX)).astype(
        ml_dtypes.float8_e4m3
    )
    xf = xf.reshape(B, 2, 128, NPIX)
    w1m = np.ascontiguousarray(w1[:, :, 0, 0].T).astype(
        ml_dtypes.float8_e4m3
    )  # [C, 32]
    nsub = QCHUNK // PCHUNK
    w2m = np.zeros((128, nsub), np.float32)               # block-diagonal
    for sb in range(nsub):
        w2m[32 * sb : 32 * (sb + 1), sb] = w2[0, :, 0, 0]
    w2m = w2m.astype(_BF16)
    b1m = np.tile(b1, nsub).reshape(128, 1).astype(np.float32)
    b2m = np.full((nsub, 1), b2[0], np.float32)
    in_maps = [
        {
            "xs": np.ascontiguousarray(xf[i * SPC : (i + 1) * SPC]),
            "w1t": w1m, "w2t": w2m, "b1p": b1m, "b2p": b2m,
        }
        for i in range(NCORES)
    ]
    res = run_bass_kernel_spmd(
        nc, in_maps, core_ids=list(range(NCORES)), trace=trace
    )
    nqq = NPIX // QCHUNK
    s = np.empty(B, np.float64)
    for i in range(NCORES):
        o = res.results[i]["out"].astype(np.float64)      # [nsub, SPC*nqq]
        for sp in range(SPC):
            s[i * SPC + sp] = o[:, sp * nqq : (sp + 1) * nqq].sum()
    s = s.astype(np.float32)
    means = 20.0 * s / NPIX + 1.0
    ksz = np.clip(np.floor(means), 1.0, float(K))
    rad = np.floor((ksz - 1.0) / 2.0).astype(np.int64)
    return rad, res.exec_time_ns


# ---------------------------------------------------------------- NEFF-B


def _build_toeplitz(kern, kjs):
    """kern: [C, K, K] masked kernel (f32). Returns (T1, T2, tail_sizes).

    T1[c, j, hp, h]        = kern[c, hp-h, kjs[j]]      (hp in [0,128))
    T2[c, t, s*20+d2, h]   = kern[c, 128+d2-h, Jt[s]]   (hp = 128+d2)
    """
    nkj = len(kjs)
    T1 = np.zeros((C, nkj, 128, H), np.float32)
    for d in range(K):
        h_idx = np.arange(0, 128 - d)
        T1[:, :, h_idx + d, h_idx] = kern[:, d, kjs][:, :, None]
    ntile = (nkj + TAIL_PACK - 1) // TAIL_PACK
    T2 = np.zeros((C, ntile, TROWS, H), np.float32)
    tail_sizes = []
    for t in range(ntile):
        jt = kjs[TAIL_PACK * t : TAIL_PACK * (t + 1)]
        tail_sizes.append(len(jt))
        for s, kj in enumerate(jt):
            for d2 in range(20):
                for ki in range(d2 + 1, K):
                    h = 128 + d2 - ki
                    if 0 <= h < H:
                        T2[:, t, s * 20 + d2, h] = kern[:, ki, kj]
    return T1, T2, tail_sizes


def _build_conv(groups, nkj_tot, nt2_tot):
    """groups: list of (b0, nsamp, kjs, tail_sizes, t1_off, t2_off)."""
    import concourse.bass as bass
    import concourse.mybir as mybir
    import concourse.tile as tile

    dt = mybir.dt
    nc = bass.Bass()
    xq = nc.declare_dram_parameter(
        "xq", [CPC, HP, B, HP], dt.bfloat16, isOutput=False
    )
    xt2 = nc.declare_dram_parameter(
        "xt2", [CPC, TROWS, nt2_tot, B, W], dt.bfloat16, isOutput=False
    )
    t1 = nc.declare_dram_parameter(
        "t1", [CPC, 128, nkj_tot, H], dt.bfloat16, isOutput=False
    )
    t2 = nc.declare_dram_parameter(
        "t2", [CPC, TROWS, nt2_tot, H], dt.bfloat16, isOutput=False
    )
    outp = nc.declare_dram_parameter(
        "out", [CPC, H, B, W], dt.float32, isOutput=True
    )

    with tile.TileContext(nc) as tc:
        with (
            tc.tile_pool(name="xa", bufs=3) as xapool,
            tc.tile_pool(name="w1", bufs=3) as w1pool,
            tc.tile_pool(name="w2", bufs=3) as w2pool,
            tc.tile_pool(name="xr", bufs=3) as xrpool,
            tc.tile_pool(name="ot", bufs=3) as opool,
            tc.tile_pool(name="ps", bufs=2, space="PSUM") as pspool,
        ):
            for c in range(CPC):
                xa = xapool.tile([128, B, HP], dt.bfloat16)
                nc.sync.dma_start(xa[:], xq[c, 0:128])
                xr = xrpool.tile([TROWS, nt2_tot, B, W], dt.bfloat16)
                nc.sync.dma_start(xr[:], xt2[c])
                tw1 = w1pool.tile([128, nkj_tot, H], dt.bfloat16)
                nc.sync.dma_start(tw1[:], t1[c])
                tw2 = w2pool.tile([TROWS, nt2_tot, H], dt.bfloat16)
                nc.sync.dma_start(tw2[:], t2[c])

                for (b0, nsamp, kjs, tails, o1, o2) in groups:
                    pt = pspool.tile([128, nsamp, W], dt.float32)
                    ot = opool.tile([128, nsamp, W], dt.float32)
                    nbank = (nsamp + 3) // 4
                    last_t = len(tails) - 1
                    for j, kj in enumerate(kjs):
                        for nb in range(nbank):
                            bs = nb * 4
                            be = min(bs + 4, nsamp)
                            nc.tensor.matmul(
                                pt[:, bs:be, :],
                                tw1[:, o1 + j, :],
                                xa[:, b0 + bs : b0 + be, kj : kj + W],
                                start=(j == 0),
                                stop=False,
                            )
                    for t, tsz in enumerate(tails):
                        rows = 20 * tsz
                        for nb in range(nbank):
                            bs = nb * 4
                            be = min(bs + 4, nsamp)
                            nc.tensor.matmul(
                                pt[:, bs:be, :],
                                tw2[0:rows, o2 + t, :],
                                xr[0:rows, o2 + t, b0 + bs : b0 + be, :],
                                start=False,
                                stop=(t == last_t),
                            )
                    nc.scalar.copy(ot[:], pt[:])
                    nc.sync.dma_start(
                        outp[c, :, b0 : b0 + nsamp, :], ot[:]
                    )
    return _split_multiwaits(nc)


def kernel(**inputs):
    x = np.asarray(inputs["x"], np.float32)
    gauss_kernel = np.asarray(inputs["gauss_kernel"], np.float32)
    w1 = np.asarray(inputs["w1"], np.float32)
    b1 = np.asarray(inputs["b1"], np.float32)
    w2 = np.asarray(inputs["w2"], np.float32)
    b2 = np.asarray(inputs["b2"], np.float32)

    out, _, _ = _kernel_impl(x, gauss_kernel, w1, b1, w2, b2, trace=False)
    return out


def _kernel_impl(x, gauss_kernel, w1, b1, w2, b2, trace=False):
    from concourse.bass_utils import run_bass_kernel_spmd

    rad, ns_a = _run_pred(x, w1, b1, w2, b2, trace=trace)

    # group samples by radius (descending), contiguous after permutation
    order = np.argsort(-rad, kind="stable")
    rad_sorted = rad[order]
    groups_meta = []        # (b0, nsamp, radius)
    gb = 0
    for r in np.unique(rad_sorted)[::-1]:
        n = int((rad_sorted == r).sum())
        groups_meta.append((gb, n, int(r)))
        gb += n

    # masked kernels + Toeplitz weights per group
    coords = np.abs(np.arange(K) - P)
    t1_parts, t2_parts, groups = [], [], []
    o1 = o2 = 0
    for (gb0, gn, r) in groups_meta:
        mask = (
            (coords[:, None] <= r) & (coords[None, :] <= r)
        ).astype(np.float32)
        kern = gauss_kernel[:, 0] * mask                  # [C, K, K]
        kjs = list(range(P - r, P + r + 1))
        T1, T2, tails = _build_toeplitz(kern, kjs)
        t1_parts.append(T1)
        t2_parts.append(T2)
        groups.append((gb0, gn, kjs, tails, o1, o2))
        o1 += len(kjs)
        o2 += len(tails)
    nkj_tot, nt2_tot = o1, o2

    # DMA-friendly layouts: partition-major, contiguous free runs
    T1 = np.ascontiguousarray(
        np.concatenate(t1_parts, axis=1).transpose(0, 2, 1, 3)
    ).astype(_BF16)                                       # [C, 128, nkj, H]
    T2 = np.ascontiguousarray(
        np.concatenate(t2_parts, axis=1).transpose(0, 2, 1, 3)
    ).astype(_BF16)                                       # [C, 120, nt2, H]

    xp_pad = np.pad(
        x[order], ((0, 0), (0, 0), (P, P), (P, P)), mode="reflect"
    )                                                     # [B, C, HP, HP] f32
    xq = np.ascontiguousarray(
        xp_pad.transpose(1, 2, 0, 3)
    ).astype(_BF16)                                       # [C, HP, B, HP]

    # host-replicated tail block: xt2[c, 20s+d2, o2+t, b, w]
    #   = xp[b, c, 128+d2, w + kj(t,s)]  for b in the group
    xt2 = np.zeros((C, TROWS, nt2_tot, B, W), _BF16)
    for (gb0, gn, kjs, tails, go1, go2) in groups:
        for t, tsz in enumerate(tails):
            for s in range(tsz):
                kj = kjs[TAIL_PACK * t + s]
                for d2 in range(20):
                    xt2[:, 20 * s + d2, go2 + t, gb0 : gb0 + gn, :] = (
                        xp_pad[gb0 : gb0 + gn, :, 128 + d2, kj : kj + W]
                        .transpose(1, 0, 2)
                    )

    nc = _build_conv(groups, nkj_tot, nt2_tot)
    in_maps = [
        {
            "xq": np.ascontiguousarray(xq[i * CPC : (i + 1) * CPC]),
            "xt2": np.ascontiguousarray(xt2[i * CPC : (i + 1) * CPC]),
            "t1": np.ascontiguousarray(T1[i * CPC : (i + 1) * CPC]),
            "t2": np.ascontiguousarray(T2[i * CPC : (i + 1) * CPC]),
        }
        for i in range(NCORES)
    ]
    res = run_bass_kernel_spmd(
        nc, in_maps, core_ids=list(range(NCORES)), trace=trace
    )

    out = np.empty((B, C, H, W), np.float32)
    inv = np.empty(B, np.int64)
    inv[order] = np.arange(B)
    for i in range(NCORES):
        # res: [CPC, H, B, W] -> [B, CPC, H, W]
        out[:, i * CPC : (i + 1) * CPC] = res.results[i]["out"].transpose(
            2, 0, 1, 3
        )[inv]
    return out, ns_a, res.exec_time_ns
